# revision 9
# baseline (speedup 1.0000x reference)
"""DNDF tree (soft decision tree / dense MoE) kernel for Trainium2.

Full computation (reference):
    dprob  = sigmoid(x @ Wd.T + bd)                 [B, 63]
    routing[b, l] = prod_d (pos ? dprob[idx] : 1 - dprob[idx])   [B, 64]
    leaves = softmax(einsum('bi,loi->blo', x, Wl) + bl, axis=-1) [B, 64, O]
    out    = einsum('bl,blo->bo', routing, leaves)  [B, O]

Sharding: expert-parallel over the 64 leaves: core c owns leaves
8c..8c+7, computes partial = sum_{l in core} routing[:, l] * leaves[:, l, :]
over the FULL batch; the host sums the 8 per-core partials.

Routing trick (no gathers): with z = x@Wd.T + bd,
    log p      = -softplus(-z),   log(1-p) = -softplus(z)
    log routing[b,l] = -( softplus(-z) @ A + softplus(z) @ B )[b, l]
where A[n,l]=1 iff leaf l visits node n on the sigmoid branch and
B[n,l]=1 iff on the (1-sigmoid) branch.  Two small matmuls + exp.

Matmuls run as float32r (full PE rate at moving free dim >= 256,
near-fp32 precision).  Softmax skips max-subtraction (logits are
~N(0, 0.65^2); exp is safe in fp32) and fuses exp+row-sum into a single
ScalarE activation with accum_out.
"""

import numpy as np
import sys

for _p in ("/opt/trn_rl_repo", "/opt/pypackages"):
    if _p not in sys.path:
        sys.path.append(_p)

import concourse.bass as bass  # noqa: E402
import concourse.bacc as bacc  # noqa: E402
import concourse.tile as tile  # noqa: E402
from concourse import mybir  # noqa: E402
from concourse.bass_utils import run_bass_kernel_spmd  # noqa: E402

TREE_DEPTH = 6
IN_F = 1024
OUT_F = 1024
BATCH = 2048
N_LEAVES = 64
N_NODES = 63
N_CORES = 8
E = N_LEAVES // N_CORES  # experts per core = 8
HALF = BATCH // 2  # batch processed in halves to fit SBUF
BT = HALF // 128  # batch tiles per half = 8
IC = IN_F // 128  # contraction chunks = 8
F32 = mybir.dt.float32
F32R = mybir.dt.float32r
AF = mybir.ActivationFunctionType
ALU = mybir.AluOpType


def _tree_routes(depth):
    n_leaves = 2**depth
    idx = np.zeros((n_leaves, depth), dtype=np.int32)
    pos = np.zeros((n_leaves, depth), dtype=bool)
    for leaf in range(n_leaves):
        node, index = 0, leaf
        for d in range(depth):
            idx[leaf, d] = node
            pos[leaf, d] = index % 2 == 0
            node = node * 2 + 1 + index % 2
            index >>= 1
    return idx, pos


ROUTE_IDX, ROUTE_POS = _tree_routes(TREE_DEPTH)


def _selection_matrices():
    """A[n, l]=1 iff leaf l visits node n with the sigmoid branch; B for 1-sigmoid."""
    A = np.zeros((N_NODES, N_LEAVES), dtype=np.float32)
    B = np.zeros((N_NODES, N_LEAVES), dtype=np.float32)
    for leaf in range(N_LEAVES):
        for d in range(TREE_DEPTH):
            n = ROUTE_IDX[leaf, d]
            if ROUTE_POS[leaf, d]:
                A[n, leaf] = 1.0
            else:
                B[n, leaf] = 1.0
    return A, B


A_FULL, B_FULL = _selection_matrices()


def build_nc(add_bd: bool, add_bl: bool):
    """Build + compile the single-core Bass program (same NEFF on all cores)."""
    from contextlib import ExitStack

    nc = bacc.Bacc("TRN2", target_bir_lowering=False, debug=False)

    xT_d = nc.dram_tensor("xT", [IN_F, BATCH], F32R, kind="ExternalInput")
    wdT_d = nc.dram_tensor("wdT", [IN_F, N_NODES], F32R, kind="ExternalInput")
    wl_d = nc.dram_tensor("wl", [E, IN_F, OUT_F], F32R, kind="ExternalInput")
    a_d = nc.dram_tensor("amat", [N_NODES, E], F32, kind="ExternalInput")
    b_d = nc.dram_tensor("bmat", [N_NODES, E], F32, kind="ExternalInput")
    bd_d = bl_d = None
    if add_bd:
        bd_d = nc.dram_tensor("bd", [1, N_NODES], F32R, kind="ExternalInput")
    if add_bl:
        bl_d = nc.dram_tensor("bl", [E, OUT_F], F32R, kind="ExternalInput")
    out_d = nc.dram_tensor("out", [BATCH, OUT_F], F32, kind="ExternalOutput")

    def mm(out, lhsT, rhs, start, stop):
        nc.tensor.matmul(out, lhsT, rhs, start=start, stop=stop)

    with ExitStack() as ctx:
        tc = ctx.enter_context(tile.TileContext(nc))
        consts = ctx.enter_context(tc.tile_pool(name="consts", bufs=1))
        xp = ctx.enter_context(tc.tile_pool(name="xp", bufs=2))
        wp = ctx.enter_context(tc.tile_pool(name="wp", bufs=2))
        accp = ctx.enter_context(tc.tile_pool(name="accp", bufs=9))
        expp = ctx.enter_context(tc.tile_pool(name="expp", bufs=3))
        rpool = ctx.enter_context(tc.tile_pool(name="rpool", bufs=18))
        spp = ctx.enter_context(tc.tile_pool(name="spp", bufs=1))
        smallp = ctx.enter_context(tc.tile_pool(name="smallp", bufs=6))
        zps = ctx.enter_context(tc.tile_pool(name="zps", bufs=1, space="PSUM"))
        rps = ctx.enter_context(tc.tile_pool(name="rps", bufs=2, space="PSUM"))
        lps = ctx.enter_context(tc.tile_pool(name="lps", bufs=2, space="PSUM"))

        wd_t = consts.tile([128, IC, N_NODES], F32R)
        for c in range(IC):
            nc.sync.dma_start(out=wd_t[:, c, :], in_=wdT_d[c * 128 : (c + 1) * 128, :])
        a_t = consts.tile([N_NODES, E], F32)
        nc.sync.dma_start(out=a_t, in_=a_d[:])
        b_t = consts.tile([N_NODES, E], F32)
        nc.sync.dma_start(out=b_t, in_=b_d[:])
        ones_t = bd_t = bl_t = None
        if add_bd or add_bl:
            ones_t = consts.tile([1, 512], F32R)
            nc.vector.memset(ones_t, 1.0)
        if add_bd:
            bd_t = consts.tile([1, N_NODES], F32R)
            nc.sync.dma_start(out=bd_t, in_=bd_d[:])

        for h in range(2):
            x_t = xp.tile([128, IC, HALF], F32R, tag="x")
            for c in range(IC):
                nc.sync.dma_start(
                    out=x_t[:, c, :],
                    in_=xT_d[c * 128 : (c + 1) * 128, h * HALF : (h + 1) * HALF],
                )

            # --- routing for this half ---
            z_ps = zps.tile([N_NODES, HALF], F32, tag="z")
            for bc in range(HALF // 512):
                zsl = z_ps[:, bc * 512 : (bc + 1) * 512]
                for c in range(IC):
                    mm(
                        zsl,
                        wd_t[:, c, :],
                        x_t[:, c, bc * 512 : (bc + 1) * 512],
                        start=(c == 0),
                        stop=(c == IC - 1 and not add_bd),
                    )
                if add_bd:
                    mm(zsl, bd_t[0:1, :], ones_t[0:1, :], start=False, stop=True)
            # softplus(t) = Ln(Exp(t) + 1) — keeps everything on one ACT
            # table set (natural_log_exp_and_others); Softplus has no table.
            sp_pos = spp.tile([N_NODES, HALF], F32, tag="sp_pos")
            nc.scalar.activation(sp_pos, z_ps, AF.Exp, scale=1.0)
            nc.scalar.activation(sp_pos, sp_pos, AF.Ln, bias=1.0)
            sp_neg = spp.tile([N_NODES, HALF], F32, tag="sp_neg")
            nc.scalar.activation(sp_neg, z_ps, AF.Exp, scale=-1.0)
            nc.scalar.activation(sp_neg, sp_neg, AF.Ln, bias=1.0)
            r_ts = []
            for bt in range(BT):
                r_ps = rps.tile([128, E], F32, tag="r_ps")
                nc.tensor.matmul(
                    r_ps, sp_neg[:, bt * 128 : (bt + 1) * 128], a_t,
                    start=True, stop=False,
                )
                nc.tensor.matmul(
                    r_ps, sp_pos[:, bt * 128 : (bt + 1) * 128], b_t,
                    start=False, stop=True,
                )
                r_t = rpool.tile([128, E], F32, tag="r", name=f"r_{h}_{bt}")
                nc.scalar.activation(r_t, r_ps, AF.Exp, scale=-1.0)
                r_ts.append(r_t)

            # --- experts ---
            accs = [
                accp.tile([128, OUT_F], F32, tag="acc", name=f"acc_{h}_{bt}")
                for bt in range(BT)
            ]
            for e in range(E):
                w_t = wp.tile([128, IC, OUT_F], F32R, tag="w", name=f"w_{h}_{e}")
                for c in range(IC):
                    nc.sync.dma_start(
                        out=w_t[:, c, :], in_=wl_d[e, c * 128 : (c + 1) * 128, :]
                    )
                ble_t = None
                if add_bl:
                    ble_t = smallp.tile([1, OUT_F], F32R, tag="ble", bufs=2)
                    nc.sync.dma_start(out=ble_t, in_=bl_d[e : e + 1, :])
                for bt in range(BT):
                    lp = lps.tile([128, OUT_F], F32, tag="lp")
                    for c in range(IC):
                        for oc in range(2):
                            mm(
                                lp[:, oc * 512 : (oc + 1) * 512],
                                x_t[:, c, bt * 128 : (bt + 1) * 128],
                                w_t[:, c, oc * 512 : (oc + 1) * 512],
                                start=(c == 0),
                                stop=(c == IC - 1 and not add_bl),
                            )
                    if add_bl:
                        for oc in range(2):
                            mm(
                                lp[:, oc * 512 : (oc + 1) * 512],
                                ones_t[0:1, 0:128],
                                ble_t[0:1, oc * 512 : (oc + 1) * 512],
                                start=False,
                                stop=True,
                            )
                    exp_t = expp.tile([128, OUT_F], F32, tag="exp")
                    s_t = smallp.tile([128, 1], F32, tag="s")
                    nc.scalar.activation(exp_t, lp, AF.Exp, accum_out=s_t)
                    sc_t = smallp.tile([128, 1], F32, tag="sc")
                    nc.vector.reciprocal(sc_t, s_t)
                    sc2_t = smallp.tile([128, 1], F32, tag="sc2")
                    nc.vector.tensor_scalar_mul(sc2_t, sc_t, r_ts[bt][:, e : e + 1])
                    if e == 0:
                        nc.vector.tensor_scalar_mul(accs[bt], exp_t, sc2_t)
                    else:
                        nc.vector.scalar_tensor_tensor(
                            accs[bt], exp_t, sc2_t, accs[bt], op0=ALU.mult, op1=ALU.add
                        )
            for bt in range(BT):
                nc.sync.dma_start(
                    out=out_d[h * HALF + bt * 128 : h * HALF + (bt + 1) * 128, :],
                    in_=accs[bt],
                )

    nc.compile()
    return nc


def make_core_inputs(x, Wd, bd, Wl, bl, core, add_bd, add_bl):
    lo, hi = core * E, (core + 1) * E
    m = {
        "xT": np.ascontiguousarray(x.T.astype(np.float32)),
        "wdT": np.ascontiguousarray(Wd.T.astype(np.float32)),
        "wl": np.ascontiguousarray(Wl[lo:hi].transpose(0, 2, 1).astype(np.float32)),
        "amat": np.ascontiguousarray(A_FULL[:, lo:hi]),
        "bmat": np.ascontiguousarray(B_FULL[:, lo:hi]),
    }
    if add_bd:
        m["bd"] = np.ascontiguousarray(bd.astype(np.float32)).reshape(1, N_NODES)
    if add_bl:
        m["bl"] = np.ascontiguousarray(bl[lo:hi].astype(np.float32))
    return m


_NC_CACHE = {}


def _get_nc(add_bd, add_bl):
    key = (add_bd, add_bl)
    if key not in _NC_CACHE:
        _NC_CACHE[key] = build_nc(add_bd, add_bl)
    return _NC_CACHE[key]


def run_spmd(x, Wd, bd, Wl, bl, trace=False):
    add_bd = bool(np.any(bd))
    add_bl = bool(np.any(bl))
    nc = _get_nc(add_bd, add_bl)
    in_maps = [
        make_core_inputs(x, Wd, bd, Wl, bl, c, add_bd, add_bl) for c in range(N_CORES)
    ]
    res = run_bass_kernel_spmd(nc, in_maps, core_ids=list(range(N_CORES)), trace=trace)
    partials = np.stack([r["out"] for r in res.results], axis=0)
    out = partials.sum(axis=0, dtype=np.float64).astype(np.float32)
    return out, res


def kernel(x, Wd, bd, Wl, bl):
    out, _ = run_spmd(
        np.asarray(x), np.asarray(Wd), np.asarray(bd), np.asarray(Wl), np.asarray(bl)
    )
    return out


# revision 18
# speedup vs baseline: 1.2317x; 1.2317x over previous
"""DNDF tree (soft decision tree / dense MoE) kernel for Trainium2.

Full computation (reference):
    dprob  = sigmoid(x @ Wd.T + bd)                 [B, 63]
    routing[b, l] = prod_d (pos ? dprob[idx] : 1 - dprob[idx])   [B, 64]
    leaves = softmax(einsum('bi,loi->blo', x, Wl) + bl, axis=-1) [B, 64, O]
    out    = einsum('bl,blo->bo', routing, leaves)  [B, O]

Sharding: expert-parallel over the 64 leaves: core c owns leaves
8c..8c+7, computes partial = sum_{l in core} routing[:, l] * leaves[:, l, :]
over the FULL batch; the host sums the 8 per-core partials.

Routing trick (no gathers): with z = x@Wd.T + bd,
    log p      = -softplus(-z),   log(1-p) = -softplus(z)
    log routing[b,l] = -( softplus(-z) @ A + softplus(z) @ B )[b, l]
where A[n,l]=1 iff leaf l visits node n on the sigmoid branch and
B[n,l]=1 iff on the (1-sigmoid) branch.  Two small matmuls + exp.
softplus(t) = Ln(Exp(t)+1) — computed with the one ACT table set that
has both exp and ln (Softplus has no table on cayman).

Matmuls run as float32r (full PE rate at moving free dim >= 256,
near-fp32 precision).  All routing work is hoisted to the kernel start so
the 2048-matmul expert stream runs back-to-back on the PE.  Softmax skips
max-subtraction (logits are ~N(0, 0.65^2); exp is safe in fp32) and fuses
exp+row-sum into a single ScalarE activation with accum_out.
"""

import numpy as np
import sys

for _p in ("/opt/trn_rl_repo", "/opt/pypackages"):
    if _p not in sys.path:
        sys.path.append(_p)

import concourse.bass as bass  # noqa: E402,F401
import concourse.bacc as bacc  # noqa: E402
import concourse.tile as tile  # noqa: E402
from concourse import mybir  # noqa: E402
from concourse.bass_utils import run_bass_kernel_spmd  # noqa: E402

TREE_DEPTH = 6
IN_F = 1024
OUT_F = 1024
BATCH = 2048
N_LEAVES = 64
N_NODES = 63
N_CORES = 8
E = N_LEAVES // N_CORES  # experts per core = 8
IC = IN_F // 128  # contraction chunks = 8
F32 = mybir.dt.float32
F32R = mybir.dt.float32r
BF16 = mybir.dt.bfloat16
F16 = mybir.dt.float16
AF = mybir.ActivationFunctionType
ALU = mybir.AluOpType


def _tree_routes(depth):
    n_leaves = 2**depth
    idx = np.zeros((n_leaves, depth), dtype=np.int32)
    pos = np.zeros((n_leaves, depth), dtype=bool)
    for leaf in range(n_leaves):
        node, index = 0, leaf
        for d in range(depth):
            idx[leaf, d] = node
            pos[leaf, d] = index % 2 == 0
            node = node * 2 + 1 + index % 2
            index >>= 1
    return idx, pos


ROUTE_IDX, ROUTE_POS = _tree_routes(TREE_DEPTH)


def _selection_matrices():
    """A[n, l]=1 iff leaf l visits node n with the sigmoid branch; B for 1-sigmoid."""
    A = np.zeros((N_NODES, N_LEAVES), dtype=np.float32)
    B = np.zeros((N_NODES, N_LEAVES), dtype=np.float32)
    for leaf in range(N_LEAVES):
        for d in range(TREE_DEPTH):
            n = ROUTE_IDX[leaf, d]
            if ROUTE_POS[leaf, d]:
                A[n, leaf] = 1.0
            else:
                B[n, leaf] = 1.0
    return A, B


A_FULL, B_FULL = _selection_matrices()

# float16 is the default: 10-bit mantissa keeps the end-to-end error at
# ~3e-4 (vs 2.5e-4 for fp32r, 2e-3 for bf16) while streaming at the full
# 1 col/cycle PE rate with fast (FWL) weight loads; half-width x/w also fit
# the whole batch in SBUF in one pass.  fp32r needs 2 batch passes (weights
# stream twice) and its 4-byte weight loads pace the PE at 227ns/MM.
MMDT = "f16"  # "f32r" | "bf16" | "f16"


def build_nc(add_bd: bool, add_bl: bool, mmdt: str = MMDT):
    """Build + compile the single-core Bass program (same NEFF on all cores)."""
    from contextlib import ExitStack

    MDT = {"f32r": F32R, "bf16": BF16, "f16": F16}[mmdt]
    NH = 2 if mmdt == "f32r" else 1  # batch passes
    HALF = BATCH // NH
    BT = HALF // 128  # batch tiles per pass

    nc = bacc.Bacc("TRN2", target_bir_lowering=False, debug=False)

    xT_d = nc.dram_tensor("xT", [IN_F, BATCH], MDT, kind="ExternalInput")
    wdT_d = nc.dram_tensor("wdT", [IN_F, N_NODES], MDT, kind="ExternalInput")
    wl_d = nc.dram_tensor("wl", [E, IN_F, OUT_F], MDT, kind="ExternalInput")
    a_d = nc.dram_tensor("amat", [N_NODES, E], F32, kind="ExternalInput")
    b_d = nc.dram_tensor("bmat", [N_NODES, E], F32, kind="ExternalInput")
    bd_d = bl_d = None
    if add_bd:
        bd_d = nc.dram_tensor("bd", [1, N_NODES], MDT, kind="ExternalInput")
    if add_bl:
        bl_d = nc.dram_tensor("bl", [E, OUT_F], MDT, kind="ExternalInput")
    out_d = nc.dram_tensor("out", [BATCH, OUT_F], F32, kind="ExternalOutput")

    mm = lambda out, lhsT, rhs, start, stop: nc.tensor.matmul(  # noqa: E731
        out, lhsT, rhs, start=start, stop=stop
    )

    with ExitStack() as ctx:
        tc = ctx.enter_context(tile.TileContext(nc))
        consts = ctx.enter_context(tc.tile_pool(name="consts", bufs=1))
        xp = ctx.enter_context(tc.tile_pool(name="xp", bufs=NH))
        wp = ctx.enter_context(tc.tile_pool(name="wp", bufs=2))
        accp = ctx.enter_context(tc.tile_pool(name="accp", bufs=BT + (NH > 1)))
        expp = ctx.enter_context(tc.tile_pool(name="expp", bufs=3))
        rpool = ctx.enter_context(tc.tile_pool(name="rpool", bufs=17))
        spp = ctx.enter_context(tc.tile_pool(name="spp", bufs=NH))
        smallp = ctx.enter_context(tc.tile_pool(name="smallp", bufs=6))
        zps = ctx.enter_context(tc.tile_pool(name="zps", bufs=2, space="PSUM"))
        rps = ctx.enter_context(tc.tile_pool(name="rps", bufs=2, space="PSUM"))
        lps = ctx.enter_context(tc.tile_pool(name="lps", bufs=2, space="PSUM"))

        wd_t = consts.tile([128, IC, N_NODES], MDT)
        for c in range(IC):
            nc.sync.dma_start(out=wd_t[:, c, :], in_=wdT_d[c * 128 : (c + 1) * 128, :])
        a_t = consts.tile([N_NODES, E], F32)
        nc.sync.dma_start(out=a_t, in_=a_d[:])
        b_t = consts.tile([N_NODES, E], F32)
        nc.sync.dma_start(out=b_t, in_=b_d[:])
        ones_t = bd_t = None
        if add_bd or add_bl:
            ones_t = consts.tile([1, 512], MDT)
            nc.vector.memset(ones_t, 1.0)
        if add_bd:
            bd_t = consts.tile([1, N_NODES], MDT)
            nc.sync.dma_start(out=bd_t, in_=bd_d[:])

        # DMA pieces of 512 fp32 columns spread transfers over all 16 queues
        # (per-queue bandwidth, not aggregate HBM, is the latency limit).
        def dma_x(h):
            x_t = xp.tile([128, IC, HALF], MDT, tag="x", name=f"x_{h}")
            for c in range(IC):
                for p in range(HALF // 512):
                    nc.sync.dma_start(
                        out=x_t[:, c, p * 512 : (p + 1) * 512],
                        in_=xT_d[
                            c * 128 : (c + 1) * 128,
                            h * HALF + p * 512 : h * HALF + (p + 1) * 512,
                        ],
                    )
            return x_t

        def dma_w(h, e):
            w_t = wp.tile([128, IC, OUT_F], MDT, tag="w", name=f"w_{h}_{e}")
            for c in range(IC):
                for p in range(2):
                    nc.sync.dma_start(
                        out=w_t[:, c, p * 512 : (p + 1) * 512],
                        in_=wl_d[e, c * 128 : (c + 1) * 128, p * 512 : (p + 1) * 512],
                    )
            return w_t

        r_ts = {}

        def routing(h, x_t):
            sp_pos = spp.tile([N_NODES, HALF], F32, tag="sp_pos", name=f"sp_p_{h}")
            sp_neg = spp.tile([N_NODES, HALF], F32, tag="sp_neg", name=f"sp_n_{h}")
            for bc in range(HALF // 512):
                sl = slice(bc * 512, (bc + 1) * 512)
                z_ps = zps.tile([N_NODES, 512], F32, tag="z")
                for c in range(IC):
                    mm(
                        z_ps,
                        wd_t[:, c, :],
                        x_t[:, c, sl],
                        start=(c == 0),
                        stop=(c == IC - 1 and not add_bd),
                    )
                if add_bd:
                    mm(z_ps, bd_t[0:1, :], ones_t[0:1, :], start=False, stop=True)
                # softplus(z) and softplus(-z) via exp + ln(x+1)
                nc.scalar.activation(sp_pos[:, sl], z_ps, AF.Exp, scale=1.0)
                nc.scalar.activation(sp_pos[:, sl], sp_pos[:, sl], AF.Ln, bias=1.0)
                nc.scalar.activation(sp_neg[:, sl], z_ps, AF.Exp, scale=-1.0)
                nc.scalar.activation(sp_neg[:, sl], sp_neg[:, sl], AF.Ln, bias=1.0)
            for bt in range(BT):
                r_ps = rps.tile([128, E], F32, tag="r_ps")
                nc.tensor.matmul(
                    r_ps,
                    sp_neg[:, bt * 128 : (bt + 1) * 128],
                    a_t,
                    start=True,
                    stop=False,
                )
                nc.tensor.matmul(
                    r_ps,
                    sp_pos[:, bt * 128 : (bt + 1) * 128],
                    b_t,
                    start=False,
                    stop=True,
                )
                r_t = rpool.tile([128, E], F32, tag="r", name=f"r_{h}_{bt}")
                nc.scalar.activation(r_t, r_ps, AF.Exp, scale=-1.0)
                r_ts[(h, bt)] = r_t

        s0_ts = {}

        def expert(h, e, x_t, w_t, accs):
            ble_t = None
            if add_bl:
                ble_t = smallp.tile([1, OUT_F], MDT, tag="ble", bufs=2)
                nc.sync.dma_start(out=ble_t, in_=bl_d[e : e + 1, :])
            for bt in range(BT):
                lp = lps.tile([128, OUT_F], F32, tag="lp")
                for c in range(IC):
                    for oc in range(2):
                        mm(
                            lp[:, oc * 512 : (oc + 1) * 512],
                            x_t[:, c, bt * 128 : (bt + 1) * 128],
                            w_t[:, c, oc * 512 : (oc + 1) * 512],
                            start=(c == 0),
                            stop=(c == IC - 1 and not add_bl),
                        )
                if add_bl:
                    for oc in range(2):
                        mm(
                            lp[:, oc * 512 : (oc + 1) * 512],
                            ones_t[0:1, 0:128],
                            ble_t[0:1, oc * 512 : (oc + 1) * 512],
                            start=False,
                            stop=True,
                        )
                if e == 0:
                    # Pure-ScalarE eviction: park unscaled exp in the
                    # accumulator; the routing/softmax scale is applied at
                    # e==1, so expert 0 runs before routing even exists
                    # (the PE warms up on it while routing computes behind).
                    s0 = smallp.tile([128, 1], F32, tag="s0", bufs=BT + 2,
                                     name=f"s0_{h}_{bt}")
                    nc.scalar.activation(accs[bt], lp, AF.Exp, accum_out=s0)
                    s0_ts[(h, bt)] = s0
                    continue
                if e == 1:
                    # deferred scale of expert 0's parked contribution
                    d_t = smallp.tile([128, 1], F32, tag="d")
                    nc.vector.reciprocal(d_t, s0_ts[(h, bt)])
                    d2_t = smallp.tile([128, 1], F32, tag="d2")
                    nc.vector.tensor_scalar_mul(
                        d2_t, d_t, r_ts[(h, bt)][:, 0:1]
                    )
                    nc.vector.tensor_scalar_mul(accs[bt], accs[bt], d2_t)
                exp_t = expp.tile([128, OUT_F], F32, tag="exp")
                s_t = smallp.tile([128, 1], F32, tag="s")
                nc.scalar.activation(exp_t, lp, AF.Exp, accum_out=s_t)
                sc_t = smallp.tile([128, 1], F32, tag="sc")
                nc.vector.reciprocal(sc_t, s_t)
                sc2_t = smallp.tile([128, 1], F32, tag="sc2")
                nc.vector.tensor_scalar_mul(sc2_t, sc_t, r_ts[(h, bt)][:, e : e + 1])
                nc.vector.scalar_tensor_tensor(
                    accs[bt], exp_t, sc2_t, accs[bt], op0=ALU.mult, op1=ALU.add
                )
                if e == E - 1:
                    # split the writeback over 4 queues to shorten the tail
                    for p in range(4):
                        r0 = h * HALF + bt * 128
                        nc.sync.dma_start(
                            out=out_d[r0 : r0 + 128, p * 256 : (p + 1) * 256],
                            in_=accs[bt][:, p * 256 : (p + 1) * 256],
                        )

        def alloc_accs(h):
            return [
                accp.tile([128, OUT_F], F32, tag="acc", name=f"acc_{h}_{bt}")
                for bt in range(BT)
            ]

        # ---- emission order tuned so the PE never waits after startup ----
        x0 = dma_x(0)
        w00 = dma_w(0, 0)
        accs0 = alloc_accs(0)
        routing(0, x0)
        expert(0, 0, x0, w00, accs0)  # e==0 eviction is ScalarE-only
        if NH > 1:
            x1 = dma_x(1)
            w01 = dma_w(0, 1)
            expert(0, 1, x0, w01, accs0)
            routing(1, x1)  # PE does these ~5us of small MMs mid-stream
            for e in range(2, E):
                expert(0, e, x0, dma_w(0, e), accs0)
            accs1 = alloc_accs(1)
            for e in range(E):
                expert(1, e, x1, dma_w(1, e), accs1)
        else:
            for e in range(1, E):
                expert(0, e, x0, dma_w(0, e), accs0)

    nc.compile()
    return nc


def make_core_inputs(x, Wd, bd, Wl, bl, core, add_bd, add_bl, mmdt: str = MMDT):
    import ml_dtypes

    ndt = {"f32r": np.float32, "bf16": ml_dtypes.bfloat16, "f16": np.float16}[mmdt]
    lo, hi = core * E, (core + 1) * E
    m = {
        "xT": np.ascontiguousarray(x.T.astype(ndt)),
        "wdT": np.ascontiguousarray(Wd.T.astype(ndt)),
        "wl": np.ascontiguousarray(Wl[lo:hi].transpose(0, 2, 1).astype(ndt)),
        "amat": np.ascontiguousarray(A_FULL[:, lo:hi]),
        "bmat": np.ascontiguousarray(B_FULL[:, lo:hi]),
    }
    if add_bd:
        m["bd"] = np.ascontiguousarray(bd.astype(ndt)).reshape(1, N_NODES)
    if add_bl:
        m["bl"] = np.ascontiguousarray(bl[lo:hi].astype(ndt))
    return m


_NC_CACHE = {}


def _get_nc(add_bd, add_bl, mmdt: str = MMDT):
    key = (add_bd, add_bl, mmdt)
    if key not in _NC_CACHE:
        _NC_CACHE[key] = build_nc(add_bd, add_bl, mmdt)
    return _NC_CACHE[key]


def run_spmd(x, Wd, bd, Wl, bl, trace=False, mmdt: str = MMDT):
    add_bd = bool(np.any(bd))
    add_bl = bool(np.any(bl))
    nc = _get_nc(add_bd, add_bl, mmdt)
    in_maps = [
        make_core_inputs(x, Wd, bd, Wl, bl, c, add_bd, add_bl, mmdt)
        for c in range(N_CORES)
    ]
    res = run_bass_kernel_spmd(nc, in_maps, core_ids=list(range(N_CORES)), trace=trace)
    partials = np.stack([r["out"] for r in res.results], axis=0)
    out = partials.sum(axis=0, dtype=np.float64).astype(np.float32)
    return out, res


def kernel(x, Wd, bd, Wl, bl):
    out, _ = run_spmd(
        np.asarray(x), np.asarray(Wd), np.asarray(bd), np.asarray(Wl), np.asarray(bl)
    )
    return out


# revision 20
# speedup vs baseline: 1.2462x; 1.0118x over previous
"""DNDF tree (soft decision tree / dense MoE) kernel for Trainium2.

Full computation (reference):
    dprob  = sigmoid(x @ Wd.T + bd)                 [B, 63]
    routing[b, l] = prod_d (pos ? dprob[idx] : 1 - dprob[idx])   [B, 64]
    leaves = softmax(einsum('bi,loi->blo', x, Wl) + bl, axis=-1) [B, 64, O]
    out    = einsum('bl,blo->bo', routing, leaves)  [B, O]

Sharding: expert-parallel over the 64 leaves: core c owns leaves
8c..8c+7, computes partial = sum_{l in core} routing[:, l] * leaves[:, l, :]
over the FULL batch; the host sums the 8 per-core partials.

Routing trick (no gathers): with z = x@Wd.T + bd,
    log p      = -softplus(-z),   log(1-p) = -softplus(z)
    log routing[b,l] = -( softplus(-z) @ A + softplus(z) @ B )[b, l]
where A[n,l]=1 iff leaf l visits node n on the sigmoid branch and
B[n,l]=1 iff on the (1-sigmoid) branch.  Two small matmuls + exp.
softplus(t) = Ln(Exp(t)+1) — computed with the one ACT table set that
has both exp and ln (Softplus has no table on cayman).

Matmuls run as float32r (full PE rate at moving free dim >= 256,
near-fp32 precision).  All routing work is hoisted to the kernel start so
the 2048-matmul expert stream runs back-to-back on the PE.  Softmax skips
max-subtraction (logits are ~N(0, 0.65^2); exp is safe in fp32) and fuses
exp+row-sum into a single ScalarE activation with accum_out.
"""

import numpy as np
import sys

for _p in ("/opt/trn_rl_repo", "/opt/pypackages"):
    if _p not in sys.path:
        sys.path.append(_p)

import concourse.bass as bass  # noqa: E402,F401
import concourse.bacc as bacc  # noqa: E402
import concourse.tile as tile  # noqa: E402
from concourse import mybir  # noqa: E402
from concourse.bass_utils import run_bass_kernel_spmd  # noqa: E402

TREE_DEPTH = 6
IN_F = 1024
OUT_F = 1024
BATCH = 2048
N_LEAVES = 64
N_NODES = 63
N_CORES = 8
E = N_LEAVES // N_CORES  # experts per core = 8
IC = IN_F // 128  # contraction chunks = 8
F32 = mybir.dt.float32
F32R = mybir.dt.float32r
BF16 = mybir.dt.bfloat16
F16 = mybir.dt.float16
AF = mybir.ActivationFunctionType
ALU = mybir.AluOpType


def _tree_routes(depth):
    n_leaves = 2**depth
    idx = np.zeros((n_leaves, depth), dtype=np.int32)
    pos = np.zeros((n_leaves, depth), dtype=bool)
    for leaf in range(n_leaves):
        node, index = 0, leaf
        for d in range(depth):
            idx[leaf, d] = node
            pos[leaf, d] = index % 2 == 0
            node = node * 2 + 1 + index % 2
            index >>= 1
    return idx, pos


ROUTE_IDX, ROUTE_POS = _tree_routes(TREE_DEPTH)


def _selection_matrices():
    """A[n, l]=1 iff leaf l visits node n with the sigmoid branch; B for 1-sigmoid."""
    A = np.zeros((N_NODES, N_LEAVES), dtype=np.float32)
    B = np.zeros((N_NODES, N_LEAVES), dtype=np.float32)
    for leaf in range(N_LEAVES):
        for d in range(TREE_DEPTH):
            n = ROUTE_IDX[leaf, d]
            if ROUTE_POS[leaf, d]:
                A[n, leaf] = 1.0
            else:
                B[n, leaf] = 1.0
    return A, B


A_FULL, B_FULL = _selection_matrices()

# float16 is the default: 10-bit mantissa keeps the end-to-end error at
# ~3e-4 (vs 2.5e-4 for fp32r, 2e-3 for bf16) while streaming at the full
# 1 col/cycle PE rate with fast (FWL) weight loads; half-width x/w also fit
# the whole batch in SBUF in one pass.  fp32r needs 2 batch passes (weights
# stream twice) and its 4-byte weight loads pace the PE at 227ns/MM.
MMDT = "f16"  # "f32r" | "bf16" | "f16"


def build_nc(add_bd: bool, add_bl: bool, mmdt: str = MMDT):
    """Build + compile the single-core Bass program (same NEFF on all cores)."""
    from contextlib import ExitStack

    MDT = {"f32r": F32R, "bf16": BF16, "f16": F16}[mmdt]
    NH = 2 if mmdt == "f32r" else 1  # batch passes
    HALF = BATCH // NH
    BT = HALF // 128  # batch tiles per pass

    nc = bacc.Bacc("TRN2", target_bir_lowering=False, debug=False)

    xT_d = nc.dram_tensor("xT", [IN_F, BATCH], MDT, kind="ExternalInput")
    wdT_d = nc.dram_tensor("wdT", [IN_F, N_NODES], MDT, kind="ExternalInput")
    wl_d = nc.dram_tensor("wl", [E, IN_F, OUT_F], MDT, kind="ExternalInput")
    a_d = nc.dram_tensor("amat", [N_NODES, E], F32, kind="ExternalInput")
    b_d = nc.dram_tensor("bmat", [N_NODES, E], F32, kind="ExternalInput")
    bd_d = bl_d = None
    if add_bd:
        bd_d = nc.dram_tensor("bd", [1, N_NODES], MDT, kind="ExternalInput")
    if add_bl:
        bl_d = nc.dram_tensor("bl", [E, OUT_F], MDT, kind="ExternalInput")
    out_d = nc.dram_tensor("out", [BATCH, OUT_F], F32, kind="ExternalOutput")

    mm = lambda out, lhsT, rhs, start, stop: nc.tensor.matmul(  # noqa: E731
        out, lhsT, rhs, start=start, stop=stop
    )

    with ExitStack() as ctx:
        tc = ctx.enter_context(tile.TileContext(nc))
        consts = ctx.enter_context(tc.tile_pool(name="consts", bufs=1))
        xp = ctx.enter_context(tc.tile_pool(name="xp", bufs=NH))
        wp = ctx.enter_context(tc.tile_pool(name="wp", bufs=2))
        accp = ctx.enter_context(tc.tile_pool(name="accp", bufs=BT + (NH > 1)))
        expp = ctx.enter_context(tc.tile_pool(name="expp", bufs=3))
        rpool = ctx.enter_context(tc.tile_pool(name="rpool", bufs=17))
        spp = ctx.enter_context(tc.tile_pool(name="spp", bufs=NH))
        smallp = ctx.enter_context(tc.tile_pool(name="smallp", bufs=6))
        zps = ctx.enter_context(tc.tile_pool(name="zps", bufs=2, space="PSUM"))
        rps = ctx.enter_context(tc.tile_pool(name="rps", bufs=2, space="PSUM"))
        lps = ctx.enter_context(tc.tile_pool(name="lps", bufs=2, space="PSUM"))

        wd_t = consts.tile([128, IC, N_NODES], MDT)
        for c in range(IC):
            nc.sync.dma_start(out=wd_t[:, c, :], in_=wdT_d[c * 128 : (c + 1) * 128, :])
        a_t = consts.tile([N_NODES, E], F32)
        nc.sync.dma_start(out=a_t, in_=a_d[:])
        b_t = consts.tile([N_NODES, E], F32)
        nc.sync.dma_start(out=b_t, in_=b_d[:])
        ones_t = bd_t = None
        if add_bd or add_bl:
            ones_t = consts.tile([1, 512], MDT)
            nc.vector.memset(ones_t, 1.0)
        if add_bd:
            bd_t = consts.tile([1, N_NODES], MDT)
            nc.sync.dma_start(out=bd_t, in_=bd_d[:])

        # DMA pieces of 512 fp32 columns spread transfers over all 16 queues
        # (per-queue bandwidth, not aggregate HBM, is the latency limit).
        def dma_x(h):
            x_t = xp.tile([128, IC, HALF], MDT, tag="x", name=f"x_{h}")
            for c in range(IC):
                for p in range(HALF // 512):
                    nc.sync.dma_start(
                        out=x_t[:, c, p * 512 : (p + 1) * 512],
                        in_=xT_d[
                            c * 128 : (c + 1) * 128,
                            h * HALF + p * 512 : h * HALF + (p + 1) * 512,
                        ],
                    )
            return x_t

        def dma_w(h, e):
            w_t = wp.tile([128, IC, OUT_F], MDT, tag="w", name=f"w_{h}_{e}")
            for c in range(IC):
                for p in range(2):
                    nc.sync.dma_start(
                        out=w_t[:, c, p * 512 : (p + 1) * 512],
                        in_=wl_d[e, c * 128 : (c + 1) * 128, p * 512 : (p + 1) * 512],
                    )
            return w_t

        r_ts = {}

        def routing(h, x_t):
            sp_pos = spp.tile([N_NODES, HALF], F32, tag="sp_pos", name=f"sp_p_{h}")
            sp_neg = spp.tile([N_NODES, HALF], F32, tag="sp_neg", name=f"sp_n_{h}")
            for bc in range(HALF // 512):
                sl = slice(bc * 512, (bc + 1) * 512)
                z_ps = zps.tile([N_NODES, 512], F32, tag="z")
                for c in range(IC):
                    mm(
                        z_ps,
                        wd_t[:, c, :],
                        x_t[:, c, sl],
                        start=(c == 0),
                        stop=(c == IC - 1 and not add_bd),
                    )
                if add_bd:
                    mm(z_ps, bd_t[0:1, :], ones_t[0:1, :], start=False, stop=True)
                # softplus(z) and softplus(-z) via exp + ln(x+1)
                nc.scalar.activation(sp_pos[:, sl], z_ps, AF.Exp, scale=1.0)
                nc.scalar.activation(sp_pos[:, sl], sp_pos[:, sl], AF.Ln, bias=1.0)
                # softplus(-z) = softplus(z) - z (exact); DVE is idle here
                nc.vector.tensor_sub(sp_neg[:, sl], sp_pos[:, sl], z_ps)
            for bt in range(BT):
                r_ps = rps.tile([128, E], F32, tag="r_ps")
                nc.tensor.matmul(
                    r_ps,
                    sp_neg[:, bt * 128 : (bt + 1) * 128],
                    a_t,
                    start=True,
                    stop=False,
                )
                nc.tensor.matmul(
                    r_ps,
                    sp_pos[:, bt * 128 : (bt + 1) * 128],
                    b_t,
                    start=False,
                    stop=True,
                )
                r_t = rpool.tile([128, E], F32, tag="r", name=f"r_{h}_{bt}")
                nc.scalar.activation(r_t, r_ps, AF.Exp, scale=-1.0)
                r_ts[(h, bt)] = r_t

        s0_ts = {}

        def expert(h, e, x_t, w_t, accs):
            ble_t = None
            if add_bl:
                ble_t = smallp.tile([1, OUT_F], MDT, tag="ble", bufs=2)
                nc.sync.dma_start(out=ble_t, in_=bl_d[e : e + 1, :])
            for bt in range(BT):
                lp = lps.tile([128, OUT_F], F32, tag="lp")
                for c in range(IC):
                    for oc in range(2):
                        mm(
                            lp[:, oc * 512 : (oc + 1) * 512],
                            x_t[:, c, bt * 128 : (bt + 1) * 128],
                            w_t[:, c, oc * 512 : (oc + 1) * 512],
                            start=(c == 0),
                            stop=(c == IC - 1 and not add_bl),
                        )
                if add_bl:
                    for oc in range(2):
                        mm(
                            lp[:, oc * 512 : (oc + 1) * 512],
                            ones_t[0:1, 0:128],
                            ble_t[0:1, oc * 512 : (oc + 1) * 512],
                            start=False,
                            stop=True,
                        )
                if e == 0:
                    # Pure-ScalarE eviction: park unscaled exp in the
                    # accumulator; the routing/softmax scale is applied at
                    # e==1, so expert 0 runs before routing even exists
                    # (the PE warms up on it while routing computes behind).
                    s0 = smallp.tile([128, 1], F32, tag="s0", bufs=BT + 2,
                                     name=f"s0_{h}_{bt}")
                    nc.scalar.activation(accs[bt], lp, AF.Exp, accum_out=s0)
                    s0_ts[(h, bt)] = s0
                    continue
                if e == 1:
                    # deferred scale of expert 0's parked contribution
                    d_t = smallp.tile([128, 1], F32, tag="d")
                    nc.vector.reciprocal(d_t, s0_ts[(h, bt)])
                    d2_t = smallp.tile([128, 1], F32, tag="d2")
                    nc.vector.tensor_scalar_mul(
                        d2_t, d_t, r_ts[(h, bt)][:, 0:1]
                    )
                    nc.vector.tensor_scalar_mul(accs[bt], accs[bt], d2_t)
                exp_t = expp.tile([128, OUT_F], F32, tag="exp")
                s_t = smallp.tile([128, 1], F32, tag="s")
                nc.scalar.activation(exp_t, lp, AF.Exp, accum_out=s_t)
                sc_t = smallp.tile([128, 1], F32, tag="sc")
                nc.vector.reciprocal(sc_t, s_t)
                sc2_t = smallp.tile([128, 1], F32, tag="sc2")
                nc.vector.tensor_scalar_mul(sc2_t, sc_t, r_ts[(h, bt)][:, e : e + 1])
                nc.vector.scalar_tensor_tensor(
                    accs[bt], exp_t, sc2_t, accs[bt], op0=ALU.mult, op1=ALU.add
                )
                if e == E - 1:
                    # split the writeback over 4 queues to shorten the tail
                    for p in range(4):
                        r0 = h * HALF + bt * 128
                        nc.sync.dma_start(
                            out=out_d[r0 : r0 + 128, p * 256 : (p + 1) * 256],
                            in_=accs[bt][:, p * 256 : (p + 1) * 256],
                        )

        def alloc_accs(h):
            return [
                accp.tile([128, OUT_F], F32, tag="acc", name=f"acc_{h}_{bt}")
                for bt in range(BT)
            ]

        # ---- PE warm-up: ~4.5us of throwaway matmuls on the (tiny, early)
        # wd tile flips the HAM clock gate to 8/8 before the real Z matmuls
        # start, and fills the PE-idle window while x streams in.
        warm_ps = zps.tile([N_NODES, 7 * N_NODES], F32, tag="z")
        for i in range(12):
            mm(
                warm_ps,
                wd_t[:, 0, :],
                wd_t[:, 1:8, :],
                start=(i == 0),
                stop=(i == 11),
            )
        junk_t = smallp.tile([N_NODES, 1], F32, tag="junk")
        nc.vector.reduce_max(junk_t, warm_ps, axis=mybir.AxisListType.X)

        # ---- emission order tuned so the PE never waits after startup ----
        x0 = dma_x(0)
        w00 = dma_w(0, 0)
        accs0 = alloc_accs(0)
        routing(0, x0)
        expert(0, 0, x0, w00, accs0)  # e==0 eviction is ScalarE-only
        if NH > 1:
            x1 = dma_x(1)
            w01 = dma_w(0, 1)
            expert(0, 1, x0, w01, accs0)
            routing(1, x1)  # PE does these ~5us of small MMs mid-stream
            for e in range(2, E):
                expert(0, e, x0, dma_w(0, e), accs0)
            accs1 = alloc_accs(1)
            for e in range(E):
                expert(1, e, x1, dma_w(1, e), accs1)
        else:
            for e in range(1, E):
                expert(0, e, x0, dma_w(0, e), accs0)

    nc.compile()
    return nc


def make_core_inputs(x, Wd, bd, Wl, bl, core, add_bd, add_bl, mmdt: str = MMDT):
    import ml_dtypes

    ndt = {"f32r": np.float32, "bf16": ml_dtypes.bfloat16, "f16": np.float16}[mmdt]
    lo, hi = core * E, (core + 1) * E
    m = {
        "xT": np.ascontiguousarray(x.T.astype(ndt)),
        "wdT": np.ascontiguousarray(Wd.T.astype(ndt)),
        "wl": np.ascontiguousarray(Wl[lo:hi].transpose(0, 2, 1).astype(ndt)),
        "amat": np.ascontiguousarray(A_FULL[:, lo:hi]),
        "bmat": np.ascontiguousarray(B_FULL[:, lo:hi]),
    }
    if add_bd:
        m["bd"] = np.ascontiguousarray(bd.astype(ndt)).reshape(1, N_NODES)
    if add_bl:
        m["bl"] = np.ascontiguousarray(bl[lo:hi].astype(ndt))
    return m


_NC_CACHE = {}


def _get_nc(add_bd, add_bl, mmdt: str = MMDT):
    key = (add_bd, add_bl, mmdt)
    if key not in _NC_CACHE:
        _NC_CACHE[key] = build_nc(add_bd, add_bl, mmdt)
    return _NC_CACHE[key]


def run_spmd(x, Wd, bd, Wl, bl, trace=False, mmdt: str = MMDT):
    add_bd = bool(np.any(bd))
    add_bl = bool(np.any(bl))
    nc = _get_nc(add_bd, add_bl, mmdt)
    in_maps = [
        make_core_inputs(x, Wd, bd, Wl, bl, c, add_bd, add_bl, mmdt)
        for c in range(N_CORES)
    ]
    res = run_bass_kernel_spmd(nc, in_maps, core_ids=list(range(N_CORES)), trace=trace)
    partials = np.stack([r["out"] for r in res.results], axis=0)
    out = partials.sum(axis=0, dtype=np.float64).astype(np.float32)
    return out, res


def kernel(x, Wd, bd, Wl, bl):
    out, _ = run_spmd(
        np.asarray(x), np.asarray(Wd), np.asarray(bd), np.asarray(Wl), np.asarray(bl)
    )
    return out


# revision 22
# speedup vs baseline: 1.2605x; 1.0115x over previous
"""DNDF tree (soft decision tree / dense MoE) kernel for Trainium2.

Full computation (reference):
    dprob  = sigmoid(x @ Wd.T + bd)                 [B, 63]
    routing[b, l] = prod_d (pos ? dprob[idx] : 1 - dprob[idx])   [B, 64]
    leaves = softmax(einsum('bi,loi->blo', x, Wl) + bl, axis=-1) [B, 64, O]
    out    = einsum('bl,blo->bo', routing, leaves)  [B, O]

Sharding: expert-parallel over the 64 leaves: core c owns leaves
8c..8c+7, computes partial = sum_{l in core} routing[:, l] * leaves[:, l, :]
over the FULL batch; the host sums the 8 per-core partials.

Routing trick (no gathers): with z = x@Wd.T + bd,
    log p      = -softplus(-z),   log(1-p) = -softplus(z)
    log routing[b,l] = -( softplus(-z) @ A + softplus(z) @ B )[b, l]
where A[n,l]=1 iff leaf l visits node n on the sigmoid branch and
B[n,l]=1 iff on the (1-sigmoid) branch.  Two small matmuls + exp.
softplus(t) = Ln(Exp(t)+1) — computed with the one ACT table set that
has both exp and ln (Softplus has no table on cayman).

Matmuls run as float32r (full PE rate at moving free dim >= 256,
near-fp32 precision).  All routing work is hoisted to the kernel start so
the 2048-matmul expert stream runs back-to-back on the PE.  Softmax skips
max-subtraction (logits are ~N(0, 0.65^2); exp is safe in fp32) and fuses
exp+row-sum into a single ScalarE activation with accum_out.
"""

import numpy as np
import sys

for _p in ("/opt/trn_rl_repo", "/opt/pypackages"):
    if _p not in sys.path:
        sys.path.append(_p)

import concourse.bass as bass  # noqa: E402,F401
import concourse.bacc as bacc  # noqa: E402
import concourse.tile as tile  # noqa: E402
from concourse import mybir  # noqa: E402
from concourse.bass_utils import run_bass_kernel_spmd  # noqa: E402

TREE_DEPTH = 6
IN_F = 1024
OUT_F = 1024
BATCH = 2048
N_LEAVES = 64
N_NODES = 63
N_CORES = 8
E = N_LEAVES // N_CORES  # experts per core = 8
IC = IN_F // 128  # contraction chunks = 8
F32 = mybir.dt.float32
F32R = mybir.dt.float32r
BF16 = mybir.dt.bfloat16
F16 = mybir.dt.float16
AF = mybir.ActivationFunctionType
ALU = mybir.AluOpType


def _tree_routes(depth):
    n_leaves = 2**depth
    idx = np.zeros((n_leaves, depth), dtype=np.int32)
    pos = np.zeros((n_leaves, depth), dtype=bool)
    for leaf in range(n_leaves):
        node, index = 0, leaf
        for d in range(depth):
            idx[leaf, d] = node
            pos[leaf, d] = index % 2 == 0
            node = node * 2 + 1 + index % 2
            index >>= 1
    return idx, pos


ROUTE_IDX, ROUTE_POS = _tree_routes(TREE_DEPTH)


def _selection_matrices():
    """A[n, l]=1 iff leaf l visits node n with the sigmoid branch; B for 1-sigmoid."""
    A = np.zeros((N_NODES, N_LEAVES), dtype=np.float32)
    B = np.zeros((N_NODES, N_LEAVES), dtype=np.float32)
    for leaf in range(N_LEAVES):
        for d in range(TREE_DEPTH):
            n = ROUTE_IDX[leaf, d]
            if ROUTE_POS[leaf, d]:
                A[n, leaf] = 1.0
            else:
                B[n, leaf] = 1.0
    return A, B


A_FULL, B_FULL = _selection_matrices()

# float16 is the default: 10-bit mantissa keeps the end-to-end error at
# ~3e-4 (vs 2.5e-4 for fp32r, 2e-3 for bf16) while streaming at the full
# 1 col/cycle PE rate with fast (FWL) weight loads; half-width x/w also fit
# the whole batch in SBUF in one pass.  fp32r needs 2 batch passes (weights
# stream twice) and its 4-byte weight loads pace the PE at 227ns/MM.
MMDT = "f16"  # "f32r" | "bf16" | "f16"


def build_nc(add_bd: bool, add_bl: bool, mmdt: str = MMDT):
    """Build + compile the single-core Bass program (same NEFF on all cores)."""
    from contextlib import ExitStack

    MDT = {"f32r": F32R, "bf16": BF16, "f16": F16}[mmdt]
    NH = 2 if mmdt == "f32r" else 1  # batch passes
    HALF = BATCH // NH
    BT = HALF // 128  # batch tiles per pass

    nc = bacc.Bacc("TRN2", target_bir_lowering=False, debug=False)

    xT_d = nc.dram_tensor("xT", [IN_F, BATCH], MDT, kind="ExternalInput")
    wdT_d = nc.dram_tensor("wdT", [IN_F, N_NODES], MDT, kind="ExternalInput")
    wl_d = nc.dram_tensor("wl", [E, IN_F, OUT_F], MDT, kind="ExternalInput")
    a_d = nc.dram_tensor("amat", [N_NODES, E], F32, kind="ExternalInput")
    b_d = nc.dram_tensor("bmat", [N_NODES, E], F32, kind="ExternalInput")
    bd_d = bl_d = None
    if add_bd:
        bd_d = nc.dram_tensor("bd", [1, N_NODES], MDT, kind="ExternalInput")
    if add_bl:
        bl_d = nc.dram_tensor("bl", [E, OUT_F], MDT, kind="ExternalInput")
    out_d = nc.dram_tensor("out", [BATCH, OUT_F], F32, kind="ExternalOutput")

    mm = lambda out, lhsT, rhs, start, stop: nc.tensor.matmul(  # noqa: E731
        out, lhsT, rhs, start=start, stop=stop
    )

    with ExitStack() as ctx:
        tc = ctx.enter_context(tile.TileContext(nc))
        consts = ctx.enter_context(tc.tile_pool(name="consts", bufs=1))
        xp = ctx.enter_context(tc.tile_pool(name="xp", bufs=NH))
        wp = ctx.enter_context(tc.tile_pool(name="wp", bufs=2))
        accp = ctx.enter_context(tc.tile_pool(name="accp", bufs=BT + (NH > 1)))
        expp = ctx.enter_context(tc.tile_pool(name="expp", bufs=3))
        rpool = ctx.enter_context(tc.tile_pool(name="rpool", bufs=17))
        spp = ctx.enter_context(tc.tile_pool(name="spp", bufs=NH))
        smallp = ctx.enter_context(tc.tile_pool(name="smallp", bufs=6))
        zps = ctx.enter_context(tc.tile_pool(name="zps", bufs=3, space="PSUM"))
        rps = ctx.enter_context(tc.tile_pool(name="rps", bufs=2, space="PSUM"))
        lps = ctx.enter_context(tc.tile_pool(name="lps", bufs=3, space="PSUM"))

        wd_t = consts.tile([128, IC, N_NODES], MDT)
        for c in range(IC):
            nc.sync.dma_start(out=wd_t[:, c, :], in_=wdT_d[c * 128 : (c + 1) * 128, :])
        a_t = consts.tile([N_NODES, E], F32)
        nc.sync.dma_start(out=a_t, in_=a_d[:])
        b_t = consts.tile([N_NODES, E], F32)
        nc.sync.dma_start(out=b_t, in_=b_d[:])
        ones_t = bd_t = None
        if add_bd or add_bl:
            ones_t = consts.tile([1, 512], MDT)
            nc.vector.memset(ones_t, 1.0)
        if add_bd:
            bd_t = consts.tile([1, N_NODES], MDT)
            nc.sync.dma_start(out=bd_t, in_=bd_d[:])

        # DMA pieces of 512 fp32 columns spread transfers over all 16 queues
        # (per-queue bandwidth, not aggregate HBM, is the latency limit).
        def dma_x(h):
            x_t = xp.tile([128, IC, HALF], MDT, tag="x", name=f"x_{h}")
            for c in range(IC):
                for p in range(HALF // 512):
                    nc.sync.dma_start(
                        out=x_t[:, c, p * 512 : (p + 1) * 512],
                        in_=xT_d[
                            c * 128 : (c + 1) * 128,
                            h * HALF + p * 512 : h * HALF + (p + 1) * 512,
                        ],
                    )
            return x_t

        def dma_w(h, e):
            w_t = wp.tile([128, IC, OUT_F], MDT, tag="w", name=f"w_{h}_{e}")
            for c in range(IC):
                for p in range(2):
                    nc.sync.dma_start(
                        out=w_t[:, c, p * 512 : (p + 1) * 512],
                        in_=wl_d[e, c * 128 : (c + 1) * 128, p * 512 : (p + 1) * 512],
                    )
            return w_t

        r_ts = {}

        def routing(h, x_t):
            sp_pos = spp.tile([N_NODES, HALF], F32, tag="sp_pos", name=f"sp_p_{h}")
            sp_neg = spp.tile([N_NODES, HALF], F32, tag="sp_neg", name=f"sp_n_{h}")
            for bc in range(HALF // 512):
                sl = slice(bc * 512, (bc + 1) * 512)
                z_ps = zps.tile([N_NODES, 512], F32, tag="z")
                for c in range(IC):
                    mm(
                        z_ps,
                        wd_t[:, c, :],
                        x_t[:, c, sl],
                        start=(c == 0),
                        stop=(c == IC - 1 and not add_bd),
                    )
                if add_bd:
                    mm(z_ps, bd_t[0:1, :], ones_t[0:1, :], start=False, stop=True)
                # softplus(z) and softplus(-z) via exp + ln(x+1)
                nc.scalar.activation(sp_pos[:, sl], z_ps, AF.Exp, scale=1.0)
                nc.scalar.activation(sp_pos[:, sl], sp_pos[:, sl], AF.Ln, bias=1.0)
                # softplus(-z) = softplus(z) - z (exact); DVE is idle here
                nc.vector.tensor_sub(sp_neg[:, sl], sp_pos[:, sl], z_ps)
            for bt in range(BT):
                r_ps = rps.tile([128, E], F32, tag="r_ps")
                nc.tensor.matmul(
                    r_ps,
                    sp_neg[:, bt * 128 : (bt + 1) * 128],
                    a_t,
                    start=True,
                    stop=False,
                )
                nc.tensor.matmul(
                    r_ps,
                    sp_pos[:, bt * 128 : (bt + 1) * 128],
                    b_t,
                    start=False,
                    stop=True,
                )
                r_t = rpool.tile([128, E], F32, tag="r", name=f"r_{h}_{bt}")
                nc.scalar.activation(r_t, r_ps, AF.Exp, scale=-1.0)
                r_ts[(h, bt)] = r_t

        s0_ts = {}

        def expert(h, e, x_t, w_t, accs):
            ble_t = None
            if add_bl:
                ble_t = smallp.tile([1, OUT_F], MDT, tag="ble", bufs=2)
                nc.sync.dma_start(out=ble_t, in_=bl_d[e : e + 1, :])
            for bt in range(BT):
                # one-bank psum tiles (3 slots) instead of two-bank (2 slots):
                # frees a PSUM bank for the routing z pool and deepens the
                # matmul/eviction pipeline
                target = accs[bt] if e == 0 else None
                if e != 0:
                    target = expp.tile([128, OUT_F], F32, tag="exp", name="exp_t")
                s_hs = []
                for oc in range(2):
                    lp = lps.tile([128, 512], F32, tag="lp")
                    for c in range(IC):
                        mm(
                            lp,
                            x_t[:, c, bt * 128 : (bt + 1) * 128],
                            w_t[:, c, oc * 512 : (oc + 1) * 512],
                            start=(c == 0),
                            stop=(c == IC - 1 and not add_bl),
                        )
                    if add_bl:
                        mm(
                            lp,
                            ones_t[0:1, 0:128],
                            ble_t[0:1, oc * 512 : (oc + 1) * 512],
                            start=False,
                            stop=True,
                        )
                    sh = smallp.tile(
                        [128, 1], F32,
                        tag=(f"s0h{oc}" if e == 0 else f"sh{oc}"),
                        bufs=(BT + 2 if e == 0 else 6),
                        name=f"sh_{e == 0}_{oc}",
                    )
                    nc.scalar.activation(
                        target[:, oc * 512 : (oc + 1) * 512], lp, AF.Exp,
                        accum_out=sh,
                    )
                    s_hs.append(sh)
                if e == 0:
                    # Pure-ScalarE eviction: park unscaled exp in the
                    # accumulator; the routing/softmax scale is applied at
                    # e==1, so expert 0 needs no routing result.
                    s0_ts[(h, bt)] = s_hs
                    continue
                if e == 1:
                    # deferred scale of expert 0's parked contribution
                    s0s = smallp.tile([128, 1], F32, tag="s0s")
                    nc.vector.tensor_add(s0s, s0_ts[(h, bt)][0], s0_ts[(h, bt)][1])
                    d_t = smallp.tile([128, 1], F32, tag="d")
                    nc.vector.reciprocal(d_t, s0s)
                    d2_t = smallp.tile([128, 1], F32, tag="d2")
                    nc.vector.tensor_scalar_mul(
                        d2_t, d_t, r_ts[(h, bt)][:, 0:1]
                    )
                    nc.vector.tensor_scalar_mul(accs[bt], accs[bt], d2_t)
                exp_t = target
                s_t = smallp.tile([128, 1], F32, tag="s")
                nc.vector.tensor_add(s_t, s_hs[0], s_hs[1])
                sc_t = smallp.tile([128, 1], F32, tag="sc")
                nc.vector.reciprocal(sc_t, s_t)
                sc2_t = smallp.tile([128, 1], F32, tag="sc2")
                nc.vector.tensor_scalar_mul(sc2_t, sc_t, r_ts[(h, bt)][:, e : e + 1])
                nc.vector.scalar_tensor_tensor(
                    accs[bt], exp_t, sc2_t, accs[bt], op0=ALU.mult, op1=ALU.add
                )
                if e == E - 1:
                    # split the writeback over 4 queues to shorten the tail
                    for p in range(4):
                        r0 = h * HALF + bt * 128
                        nc.sync.dma_start(
                            out=out_d[r0 : r0 + 128, p * 256 : (p + 1) * 256],
                            in_=accs[bt][:, p * 256 : (p + 1) * 256],
                        )

        def alloc_accs(h):
            return [
                accp.tile([128, OUT_F], F32, tag="acc", name=f"acc_{h}_{bt}")
                for bt in range(BT)
            ]

        # ---- PE warm-up: ~4.5us of throwaway matmuls on the (tiny, early)
        # wd tile flips the HAM clock gate to 8/8 before the real Z matmuls
        # start, and fills the PE-idle window while x streams in.
        warm_ps = zps.tile([N_NODES, 7 * N_NODES], F32, tag="z")
        for i in range(12):
            mm(
                warm_ps,
                wd_t[:, 0, :],
                wd_t[:, 1:8, :],
                start=(i == 0),
                stop=(i == 11),
            )
        junk_t = smallp.tile([N_NODES, 1], F32, tag="junk")
        nc.vector.reduce_max(junk_t, warm_ps, axis=mybir.AxisListType.X)

        # ---- emission order tuned so the PE never waits after startup ----
        x0 = dma_x(0)
        w00 = dma_w(0, 0)
        accs0 = alloc_accs(0)
        routing(0, x0)
        expert(0, 0, x0, w00, accs0)  # e==0 eviction is ScalarE-only
        if NH > 1:
            x1 = dma_x(1)
            w01 = dma_w(0, 1)
            expert(0, 1, x0, w01, accs0)
            routing(1, x1)  # PE does these ~5us of small MMs mid-stream
            for e in range(2, E):
                expert(0, e, x0, dma_w(0, e), accs0)
            accs1 = alloc_accs(1)
            for e in range(E):
                expert(1, e, x1, dma_w(1, e), accs1)
        else:
            for e in range(1, E):
                expert(0, e, x0, dma_w(0, e), accs0)

    nc.compile()
    return nc


def make_core_inputs(x, Wd, bd, Wl, bl, core, add_bd, add_bl, mmdt: str = MMDT):
    import ml_dtypes

    ndt = {"f32r": np.float32, "bf16": ml_dtypes.bfloat16, "f16": np.float16}[mmdt]
    lo, hi = core * E, (core + 1) * E
    m = {
        "xT": np.ascontiguousarray(x.T.astype(ndt)),
        "wdT": np.ascontiguousarray(Wd.T.astype(ndt)),
        "wl": np.ascontiguousarray(Wl[lo:hi].transpose(0, 2, 1).astype(ndt)),
        "amat": np.ascontiguousarray(A_FULL[:, lo:hi]),
        "bmat": np.ascontiguousarray(B_FULL[:, lo:hi]),
    }
    if add_bd:
        m["bd"] = np.ascontiguousarray(bd.astype(ndt)).reshape(1, N_NODES)
    if add_bl:
        m["bl"] = np.ascontiguousarray(bl[lo:hi].astype(ndt))
    return m


_NC_CACHE = {}


def _get_nc(add_bd, add_bl, mmdt: str = MMDT):
    key = (add_bd, add_bl, mmdt)
    if key not in _NC_CACHE:
        _NC_CACHE[key] = build_nc(add_bd, add_bl, mmdt)
    return _NC_CACHE[key]


def run_spmd(x, Wd, bd, Wl, bl, trace=False, mmdt: str = MMDT):
    add_bd = bool(np.any(bd))
    add_bl = bool(np.any(bl))
    nc = _get_nc(add_bd, add_bl, mmdt)
    in_maps = [
        make_core_inputs(x, Wd, bd, Wl, bl, c, add_bd, add_bl, mmdt)
        for c in range(N_CORES)
    ]
    res = run_bass_kernel_spmd(nc, in_maps, core_ids=list(range(N_CORES)), trace=trace)
    partials = np.stack([r["out"] for r in res.results], axis=0)
    out = partials.sum(axis=0, dtype=np.float64).astype(np.float32)
    return out, res


def kernel(x, Wd, bd, Wl, bl):
    out, _ = run_spmd(
        np.asarray(x), np.asarray(Wd), np.asarray(bd), np.asarray(Wl), np.asarray(bl)
    )
    return out
